# revision 2
# baseline (speedup 1.0000x reference)
"""ApproxEMD Trainium2 kernel — data-parallel over batch across 8 NeuronCores.

Factorized form of the reference auction iteration (match is never
materialized; its contribution to the output is accumulated per step):

  P[n,m] = |p_n|^2 + |l_m|^2 - 2 p_n.l_m     (K=5 augmented matmul on device)
  per iteration with factor ef:
    E = exp(ef*P)                                           (ScalarE)
    s0[m] = sum_n E[n,m]; s1[m] = sum_n u[n]E[n,m]          (PE, lhsT=[1|u])
    denom1 = c*s0+EPS; d2 = c*s1/denom1
    bid_wt = min(c/(d2+EPS),1); alpha = c*bid_wt/denom1     (DVE, [16,128])
    t[m] = sum_n u[n](E.P)[n,m]                             (PE over G=E.P)
    S += sum_m alpha[m]*t[m]
    c = max(c - d2*bid_wt, 0)
    r[n] = sum_m E[n,m]alpha[m]     (DVE tensor_tensor_reduce, alpha bcast)
    u = max(u - u*r, 0)
  Last factor is 0 (E==1): device only computes t[m]=sum_n u[n]P[n,m];
  the remaining O(N) math runs on the host from (t, c, u).

Each core handles B/8 = 2 batches; host sums the partial results in fp64.
"""

import numpy as np

import concourse.bass as bass
import concourse.mybir as mybir
import concourse.tile as tile
from concourse import bacc
from concourse.bass_utils import run_bass_kernel_spmd

FP32 = mybir.dt.float32
BF16 = mybir.dt.bfloat16
AF = mybir.ActivationFunctionType
OP = mybir.AluOpType

B, N, D = 16, 2048, 3
NCORES = 8
BPC = B // NCORES          # batches per core
NT = N // 128              # 16 row tiles
M = N
EPS = 1e-9
EXP_FACTORS = [-(4.0 ** i) for i in range(7, -2, -1)] + [0.0]
SKIP = 0                   # all iterations matter (multi-resolution matching)


def build_program(skip=SKIP, n_batches=BPC, upto=6):
    nc = bacc.Bacc("TRN2", target_bir_lowering=False, debug=False,
                   num_devices=NCORES)
    paug_d = nc.dram_tensor("paug", [BPC, 5, N], FP32, kind="ExternalInput").ap()
    laug_d = nc.dram_tensor("laug", [BPC, 5, M], FP32, kind="ExternalInput").ap()
    sacc_d = nc.dram_tensor("sacc", [16, 128], FP32, kind="ExternalOutput").ap()
    tfin_d = nc.dram_tensor("tfin", [BPC, 16, 128], FP32, kind="ExternalOutput").ap()
    cfin_d = nc.dram_tensor("cfin", [BPC, 16, 128], FP32, kind="ExternalOutput").ap()
    ufin_d = nc.dram_tensor("ufin", [BPC, 128, NT], FP32, kind="ExternalOutput").ap()

    with tile.TileContext(nc) as tc:
        with (
            tc.tile_pool(name="pP", bufs=1) as pP,
            tc.tile_pool(name="pE", bufs=17) as pE,
            tc.tile_pool(name="pG", bufs=2) as pG,
            tc.tile_pool(name="pH", bufs=2) as pH,
            tc.tile_pool(name="pAB", bufs=2) as pAB,
            tc.tile_pool(name="pAug", bufs=1) as pAug,
            tc.tile_pool(name="pSm", bufs=1) as pSm,
            tc.tile_pool(name="pPS", bufs=2, space=bass.MemorySpace.PSUM) as pPS,
        ):
            # ---- persistent small tiles ----
            sacc16 = pSm.tile([16, 128], FP32, tag="sacc16")
            onesrow = pSm.tile([1, 128], BF16, tag="onesrow")
            ubuf = pSm.tile([128, 2, NT], BF16, tag="ubuf")  # [:,0,:]=ones [:,1,:]=u
            ucol = pSm.tile([128, NT], FP32, tag="ucol")
            rcol = pSm.tile([128, NT], FP32, tag="rcol")
            tmpU = pSm.tile([128, NT], FP32, tag="tmpU")
            c16 = pSm.tile([16, 128], FP32, tag="c16")
            s0_16 = pSm.tile([16, 128], FP32, tag="s0_16")
            s1_16 = pSm.tile([16, 128], FP32, tag="s1_16")
            t16 = pSm.tile([16, 128], FP32, tag="t16")
            bw16 = pSm.tile([16, 128], FP32, tag="bw16")
            al16 = pSm.tile([16, 128], FP32, tag="al16")
            a16b = pSm.tile([16, 128], BF16, tag="a16b")
            tmpA = pSm.tile([16, 128], FP32, tag="tmpA")
            tmpB = pSm.tile([16, 128], FP32, tag="tmpB")
            ssrow98 = pSm.tile([36, M], FP32, tag="ssrow98")  # psum bounce
            trow97 = pSm.tile([36, M], FP32, tag="trow97")
            arowb = pSm.tile([1, M], BF16, tag="arowb")

            nc.vector.memset(sacc16[:], 0.0)
            nc.vector.memset(onesrow[:], 1.0)

            for b in range(n_batches):
                # ---- per-batch state init ----
                nc.vector.memset(ubuf[:], 1.0)
                nc.vector.memset(ucol[:], 1.0)
                nc.vector.memset(c16[:], 1.0)

                augp = pAug.tile([128, N], FP32, tag="augp")
                augl = pAug.tile([128, M], FP32, tag="augl")
                for g in range(4):
                    nc.sync.dma_start(augp[32 * g:32 * g + 5, :], paug_d[b])
                    nc.sync.dma_start(augl[32 * g:32 * g + 5, :], laug_d[b])

                # ---- build P (bf16, SBUF-resident); 4 chunks packed into
                # the 4 PE row-groups run concurrently ----
                Pt = [pP.tile([128, M], BF16, tag=f"P{i}", name=f"P{i}_{b}") for i in range(NT)]
                for i in range(NT):
                    ps = pPS.tile([128, M], FP32, tag="ps")
                    for c in range(M // 512):
                        g = c % 4
                        nc.tensor.matmul(
                            ps[:, 512 * c:512 * (c + 1)],
                            augp[32 * g:32 * g + 5, 128 * i:128 * (i + 1)],
                            augl[32 * g:32 * g + 5, 512 * c:512 * (c + 1)],
                            start=True, stop=True,
                            tile_position=(32 * g, 0),
                        )
                    if i % 2 == 0:
                        nc.vector.tensor_copy(Pt[i][:], ps[:])
                    else:
                        nc.scalar.copy(Pt[i][:], ps[:])

                # ---- auction iterations ----
                for t in range(skip, len(EXP_FACTORS) - 1):
                    ef = EXP_FACTORS[t]
                    # E = exp(ef*P)
                    Et = [pE.tile([128, M], BF16, tag="E", name=f"E{i}_{b}_{t}") for i in range(NT)]
                    for i in range(NT):
                        nc.scalar.activation(Et[i][:], Pt[i][:], AF.Exp,
                                             scale=float(ef))
                    if upto < 2:
                        continue
                    # s0,s1 column sums (over n) via PE
                    ss = pPS.tile([128, M], FP32, tag="ps")
                    for i in range(NT):
                        for c in range(M // 512):
                            nc.tensor.matmul(
                                ss[0:2, 512 * c:512 * (c + 1)],
                                ubuf[:, :, i:i+1],
                                Et[i][:, 512 * c:512 * (c + 1)],
                                start=(i == 0), stop=(i == NT - 1),
                            )
                    nc.scalar.copy(ssrow98[0:2, :], ss[0:2, :])
                    nc.sync.dma_start(s0_16[:], ssrow98[0:1, :])
                    nc.sync.dma_start(s1_16[:], ssrow98[1:2, :])

                    if upto < 3:
                        continue
                    # ---- per-m vector math on [16,128] ----
                    # tmpA = rden = 1/(c*s0+EPS)
                    nc.vector.tensor_tensor(tmpA[:], c16[:], s0_16[:], OP.mult)
                    nc.vector.tensor_scalar_add(tmpA[:], tmpA[:], EPS)
                    nc.vector.reciprocal(tmpA[:], tmpA[:])
                    # tmpB = d2 = c*s1*rden
                    nc.vector.tensor_tensor(tmpB[:], c16[:], s1_16[:], OP.mult)
                    nc.vector.tensor_tensor(tmpB[:], tmpB[:], tmpA[:], OP.mult)
                    # bw16 = bid_wt = min(c/(d2+EPS), 1)
                    nc.vector.tensor_scalar_add(bw16[:], tmpB[:], EPS)
                    nc.vector.reciprocal(bw16[:], bw16[:])
                    nc.vector.tensor_tensor(bw16[:], bw16[:], c16[:], OP.mult)
                    nc.vector.tensor_scalar_min(bw16[:], bw16[:], 1.0)
                    # al16 = alpha = c*bid_wt*rden   (before c update)
                    nc.vector.tensor_tensor(al16[:], bw16[:], tmpA[:], OP.mult)
                    nc.vector.tensor_tensor(al16[:], al16[:], c16[:], OP.mult)
                    # c = max(c - d2*bid_wt, 0)
                    nc.vector.tensor_tensor(tmpB[:], tmpB[:], bw16[:], OP.mult)
                    nc.vector.tensor_tensor(c16[:], c16[:], tmpB[:], OP.subtract)
                    nc.vector.tensor_scalar_max(c16[:], c16[:], 0.0)

                    if upto < 4:
                        continue
                    # ---- alpha broadcast: [16,128] -> row -> psum [128,M] ----
                    nc.vector.tensor_copy(a16b[:], al16[:])
                    nc.sync.dma_start(arowb[:], a16b[:])
                    ab = pPS.tile([128, M], FP32, tag="ps")
                    for c in range(M // 512):
                        nc.tensor.matmul(
                            ab[:, 512 * c:512 * (c + 1)],
                            onesrow[:],
                            arowb[0:1, 512 * c:512 * (c + 1)],
                            start=True, stop=True,
                        )
                    absb = pAB.tile([128, M], BF16, tag="absb")
                    nc.scalar.copy(absb[:], ab[:])

                    if upto < 5:
                        continue
                    # ---- big elementwise + reductions ----
                    tps = pPS.tile([128, M], FP32, tag="ps")
                    for i in range(NT):
                        G = pG.tile([128, M], BF16, tag="G")
                        nc.vector.tensor_tensor(G[:], Et[i][:], Pt[i][:], OP.mult)
                        for c in range(M // 512):
                            nc.tensor.matmul(
                                tps[0:1, 512 * c:512 * (c + 1)],
                                ubuf[:, 1:2, i:i+1],
                                G[:, 512 * c:512 * (c + 1)],
                                start=(i == 0), stop=(i == NT - 1),
                            )
                        if upto < 6:
                            continue
                        H = pH.tile([128, M], BF16, tag="H")
                        if i % 2 == 0:
                            # fused product+reduce on DVE (1x)
                            nc.vector.scalar_tensor_tensor(
                                H[:], Et[i][:], 1.0, absb[:],
                                OP.mult, OP.mult,
                                accum_out=rcol[:, i:i + 1])
                        else:
                            # product on DVE (2x bf16), reduce on ScalarE
                            nc.vector.tensor_tensor(H[:], Et[i][:], absb[:],
                                                    OP.mult)
                            nc.scalar.activation(H[:], H[:], AF.Copy,
                                                 accum_out=rcol[:, i:i + 1])
                        # quad-batched u-update: u' = max(u - u*r, 0)
                        if i % 4 == 3:
                            q = i - 3
                            nc.vector.tensor_tensor(tmpU[:, q:q + 4],
                                                    ucol[:, q:q + 4],
                                                    rcol[:, q:q + 4], OP.mult)
                            nc.vector.tensor_tensor(ucol[:, q:q + 4],
                                                    ucol[:, q:q + 4],
                                                    tmpU[:, q:q + 4], OP.subtract)
                            nc.vector.tensor_scalar_max(ucol[:, q:q + 4],
                                                        ucol[:, q:q + 4], 0.0)
                            nc.vector.tensor_copy(ubuf[:, 1:2, q:q + 4],
                                                  ucol[:, q:q + 4])
                    nc.scalar.copy(trow97[0:1, :], tps[0:1, :])
                    nc.sync.dma_start(t16[:], trow97[0:1, :])

                    # S += sum_m alpha*t
                    nc.vector.tensor_tensor(tmpB[:], al16[:], t16[:], OP.mult)
                    nc.vector.tensor_tensor(sacc16[:], sacc16[:], tmpB[:], OP.add)


                # ---- final iteration (ef == 0, E == 1): t over P only ----
                tps = pPS.tile([128, M], FP32, tag="ps")
                for i in range(NT):
                    for c in range(M // 512):
                        nc.tensor.matmul(
                            tps[0:1, 512 * c:512 * (c + 1)],
                            ubuf[:, 1:2, i:i+1],
                            Pt[i][:, 512 * c:512 * (c + 1)],
                            start=(i == 0), stop=(i == NT - 1),
                        )
                nc.scalar.copy(trow97[0:1, :], tps[0:1, :])
                nc.sync.dma_start(t16[:], trow97[0:1, :])
                nc.sync.dma_start(tfin_d[b], t16[:])
                nc.sync.dma_start(cfin_d[b], c16[:])
                nc.sync.dma_start(ufin_d[b], ucol[:])

            nc.sync.dma_start(sacc_d[:], sacc16[:])

    nc.compile()
    return nc


_CACHED = None


def _get_program():
    global _CACHED
    if _CACHED is None:
        _CACHED = build_program()
    return _CACHED


def _augment(preds, labels):
    """Host-side layout prep: augmented K=5 factors so that
    paug.T @ laug == pwdist."""
    # preds/labels: [B, N, 3] float32
    pn = np.sum(preds.astype(np.float64) ** 2, axis=-1)   # [B, N]
    ln = np.sum(labels.astype(np.float64) ** 2, axis=-1)
    Bn = preds.shape[0]
    paug = np.empty((Bn, 5, N), np.float32)
    laug = np.empty((Bn, 5, M), np.float32)
    paug[:, 0:3, :] = -2.0 * np.transpose(preds, (0, 2, 1))
    paug[:, 3, :] = pn
    paug[:, 4, :] = 1.0
    laug[:, 0:3, :] = np.transpose(labels, (0, 2, 1))
    laug[:, 3, :] = 1.0
    laug[:, 4, :] = ln
    return paug, laug


def _host_final_iteration(tvec, c, u):
    """ef == 0 contribution computed on host from device outputs.
    tvec: [M] = sum_n u[n]P[n,m]; c: [M]; u: [N]  (all fp32 -> fp64)."""
    tvec = tvec.astype(np.float64)
    c = c.astype(np.float64)
    su = float(np.sum(u.astype(np.float64)))
    s0 = float(N)
    denom1 = c * s0 + EPS
    d2 = c * su / denom1
    bid_wt = np.minimum(c / (d2 + EPS), 1.0)
    alpha = c * bid_wt / denom1
    return float(np.sum(alpha * tvec))


def _make_in_maps(preds, labels):
    preds = np.asarray(preds, dtype=np.float32)
    labels = np.asarray(labels, dtype=np.float32)
    paug, laug = _augment(preds, labels)
    in_maps = []
    for core in range(NCORES):
        sl = slice(core * BPC, (core + 1) * BPC)
        in_maps.append({
            "paug": np.ascontiguousarray(paug[sl]),
            "laug": np.ascontiguousarray(laug[sl]),
        })
    return in_maps


def _finalize(results):
    total = 0.0
    for core in range(NCORES):
        out = results[core]
        total += float(np.sum(out["sacc"].astype(np.float64)))
        for b in range(BPC):
            total += _host_final_iteration(
                out["tfin"][b].reshape(-1),
                out["cfin"][b].reshape(-1),            # [16,128] -> m-major
                np.transpose(out["ufin"][b]).reshape(-1),  # [128,NT] -> n = 128j+jj? see note
            )
    return np.float32(total)


def kernel(preds, labels):
    in_maps = _make_in_maps(preds, labels)
    nc = _get_program()
    res = run_bass_kernel_spmd(nc, in_maps, core_ids=list(range(NCORES)))
    return _finalize(res.results)

